# revision 1
# baseline (speedup 1.0000x reference)
"""DOM-masked transformer layer on 8 Trainium2 NeuronCores.

Sharding: 6144 pulses -> 8 shards of 768 queries (sequence parallel).
pulse_to_dom_idx is sorted, so documents are contiguous segments (max 14
rows here; asserted <= PAD+1 = 33).  Attention is block-diagonal: each
256-query tile only attends inside its 320-key window (256 + 2*PAD).

Per-core device program (SPMD, identical on all cores):
  - qT/kT (transposed layout) and v (row layout) projections, bf16 matmuls
  - per (tile, head): S^T computed directly in [key, query] layout by PE
    (no explicit transposes), packed into one PSUM bank; one exp (ACT) and
    one multiplicative-mask op (DVE); row sums broadcast across partitions
    via a PE ones-matmul; unnormalized PV; one reciprocal + one multiply
    per head-pair normalizes while downcasting to SBUF.
    Key chunks only touch their reachable query regions (banded structure),
    relying on matmul start=True clearing the whole PSUM bank's has_written
    bits so region writes overwrite-or-accumulate correctly.
  - out-projection, residual + LayerNorm in f32 (bn_stats/bn_aggr),
    PE-transpose of the normalized activations for the FFN
  - FFN with exact GELU (bf16 matmuls, f32 accumulation), second residual
    + LayerNorm, DMA out.

Host side (numpy, cheap): shard/pad x, transpose to xT (bf16), cast
weights to bf16, and build per-tile transposed 0/1 masks packed to match
the on-chip score layout.

Timing is measured with the repo's TimelineSim cost model (no NTFF
profiling is available through the axon tunnel); wall-clock through the
tunnel is dominated by input transfer, not kernel time.
"""

import sys
import time

if "/opt/trn_rl_repo" not in sys.path:
    sys.path.insert(0, "/opt/trn_rl_repo")

import numpy as np
import ml_dtypes

import concourse.bass as bass
import concourse.mybir as mybir
import concourse.tile as tile
from concourse.masks import make_identity

# problem constants (hardcoded per contract)
N = 6144
D = 256
H = 4
DH = 64
DFF = 1024
NCORES = 8
NQ = N // NCORES            # 768 queries per core
PAD = 32                    # halo on each side of a query tile
HALO = NQ + 2 * PAD         # 832 rows of K/V context per core
NT = NQ // 128              # 6 sub-tiles of 128 per core
LN_EPS = 1e-5

F32 = mybir.dt.float32
BF16 = mybir.dt.bfloat16
AF = mybir.ActivationFunctionType

_CACHE = {}


def _split_excess_waits(nc, max_waits=1):
    """Walrus on this toolchain rejects >1 semaphore wait on one
    instruction; move the excess onto nop carriers inserted just before."""
    for fn in nc.m.functions:
        for bb in fn.blocks:
            insts = list(bb.instructions)
            need = [
                i for i in insts
                if i.sync_info and i.sync_info.on_wait
                and len(i.sync_info.on_wait) > max_waits
            ]
            if not need:
                continue
            new_list = []
            carriers = {}
            for inst in insts:
                if inst.sync_info and inst.sync_info.on_wait and len(
                    inst.sync_info.on_wait
                ) > max_waits:
                    waits = list(inst.sync_info.on_wait)
                    extra = waits[max_waits:]
                    inst.sync_info.on_wait = waits[:max_waits]
                    eng = nc.engines[inst.engine]
                    for j in range(0, len(extra), max_waits):
                        nop = eng.nop()
                        chunk = extra[j : j + max_waits]
                        import bass_rust

                        nop.ins.sync_info = bass_rust.SyncInfo(
                            on_wait=chunk, on_update=[]
                        )
                        carriers.setdefault(inst.name, []).append(nop.ins)
                    new_list.extend(carriers[inst.name])
                new_list.append(inst)
            # the nops were appended to the current bb by emission; drop them
            # from wherever they landed and keep only our ordered copy
            all_carriers = {c.name for cs in carriers.values() for c in cs}
            for fn2 in nc.m.functions:
                for bb2 in fn2.blocks:
                    if bb2 is not bb:
                        bb2.instructions = [
                            i for i in bb2.instructions
                            if i.name not in all_carriers
                        ]
            tail = [i for i in bb.instructions if i.name in all_carriers]
            keep = set(i.name for i in new_list)
            rest = [
                i for i in bb.instructions
                if i.name not in keep and i.name not in all_carriers
            ]
            assert not rest, "unexpected new instructions during split"
            del tail
            bb.instructions = new_list


def _build_bass():
    nc = bass.Bass(target_bir_lowering=False)

    xT = nc.dram_tensor("xT", [D, HALO], BF16, kind="ExternalInput")
    x_own = nc.dram_tensor("x_own", [NQ, D], F32, kind="ExternalInput")
    qkvw = nc.dram_tensor("qkvw", [D, 3 * D], BF16, kind="ExternalInput")
    outw = nc.dram_tensor("outw", [D, D], BF16, kind="ExternalInput")
    w1 = nc.dram_tensor("w1", [D, DFF], BF16, kind="ExternalInput")
    w2 = nc.dram_tensor("w2", [DFF, D], BF16, kind="ExternalInput")
    maskb = nc.dram_tensor("maskb", [NT // 2, 128, 256 + 4 * PAD], BF16, kind="ExternalInput")
    out = nc.dram_tensor("out", [NQ, D], F32, kind="ExternalOutput")

    with tile.TileContext(nc) as tc:
        with (
            tc.tile_pool(name="singles", bufs=1) as singles,
            tc.tile_pool(name="attn", bufs=5) as attn,
            tc.tile_pool(name="small", bufs=8) as small,
            tc.tile_pool(name="ps_big", bufs=4, space="PSUM") as ps_big,
            tc.tile_pool(name="ps_st", bufs=2, space="PSUM") as ps_st_pool,
            tc.tile_pool(name="ps_acc", bufs=2, space="PSUM") as ps_acc,
        ):
            # ---- constants / weights ----
            ident = singles.tile([128, 128], BF16)
            make_identity(nc, ident)
            identf = singles.tile([128, 128], F32)
            make_identity(nc, identf)
            ones_sb = singles.tile([128, 128], BF16)
            nc.vector.memset(ones_sb, 1.0)
            eps_sb = singles.tile([128, 1], F32)
            nc.vector.memset(eps_sb, LN_EPS)

            xT_sb = [
                singles.tile([128, HALO], BF16, tag=f"xT{c}", name=f"xTs{c}")
                for c in range(2)
            ]
            qkvw_sb = [
                singles.tile([128, 3 * D], BF16, tag=f"qkvw{c}", name=f"qw{c}")
                for c in range(2)
            ]
            # first matmul needs qkvw0 (weights) + xT0 (rhs): land those first
            nc.sync.dma_start(out=qkvw_sb[0], in_=qkvw[0:128, :])
            nc.sync.dma_start(out=xT_sb[0], in_=xT[0:128, :])
            nc.sync.dma_start(out=qkvw_sb[1], in_=qkvw[128:256, :])
            nc.sync.dma_start(out=xT_sb[1], in_=xT[128:256, :])
            outw_sb = []
            for c in range(2):
                t = singles.tile([128, D], BF16, tag=f"outw{c}")
                nc.scalar.dma_start(out=t, in_=outw[c * 128 : (c + 1) * 128, :])
                outw_sb.append(t)
            w1_sb = []
            for c in range(2):
                t = singles.tile([128, DFF], BF16, tag=f"w1_{c}")
                nc.scalar.dma_start(out=t, in_=w1[c * 128 : (c + 1) * 128, :])
                w1_sb.append(t)
            w2_sb = singles.tile([128, 8, D], BF16)
            nc.scalar.dma_start(
                out=w2_sb, in_=w2.rearrange("(c p) n -> p c n", p=128)
            )

            # ---- projections: qT, kT (T layout), v (normal layout) ----
            # qT[dq, n] = sum_d Wq[d, dq] * xT[d, n]
            qT_sb = [singles.tile([128, HALO], BF16, tag=f"qT{c}", name=f"qT{c}") for c in range(2)]
            kT_sb = [singles.tile([128, HALO], BF16, tag=f"kT{c}", name=f"kT{c}") for c in range(2)]
            NCH = HALO // 2  # free-dim chunk for projection matmuls
            v_sb = singles.tile([128, 7, D], BF16)

            def emit_qkT(nchunk):
                for oc in range(4):  # dq 0:128,128:256, dk 0:128,128:256
                    dst = qT_sb[oc] if oc < 2 else kT_sb[oc - 2]
                    wcols = slice(oc * 128, (oc + 1) * 128)
                    ncols = slice(nchunk * NCH, (nchunk + 1) * NCH)
                    ps = ps_big.tile([128, NCH], F32, tag="psA", name="ps")
                    nc.tensor.matmul(
                        ps, qkvw_sb[0][:, wcols], xT_sb[0][:, ncols],
                        start=True, stop=False,
                    )
                    nc.tensor.matmul(
                        ps, qkvw_sb[1][:, wcols], xT_sb[1][:, ncols],
                        start=False, stop=True,
                    )
                    if (oc + nchunk) % 2 == 0:
                        nc.vector.tensor_copy(out=dst[:, ncols], in_=ps)
                    else:
                        nc.scalar.copy(out=dst[:, ncols], in_=ps)

            def emit_v(chunks):
                # v[n, dv] = sum_d xT[d, n]^T Wv[d, dv], normal layout chunks
                for c in chunks:
                    rows = min(128, HALO - c * 128)
                    ncols = slice(c * 128, c * 128 + rows)
                    ps = ps_big.tile([128, D], F32, tag="psA", name="ps")
                    nc.tensor.matmul(
                        ps[:rows], xT_sb[0][:, ncols],
                        qkvw_sb[0][:, 2 * D : 3 * D], start=True, stop=False,
                    )
                    nc.tensor.matmul(
                        ps[:rows], xT_sb[1][:, ncols],
                        qkvw_sb[1][:, 2 * D : 3 * D], start=False, stop=True,
                    )
                    if c % 2 == 0:
                        nc.vector.tensor_copy(out=v_sb[:rows, c, :], in_=ps[:rows])
                    else:
                        nc.scalar.copy(out=v_sb[:rows, c, :], in_=ps[:rows])

            # order: everything tile T=0 needs first, the rest behind it
            emit_qkT(0)
            emit_v(range(0, 3))
            emit_qkT(1)
            emit_v(range(3, 7))

            # ---- attention + rest, per query tile ----
            x2_all = singles.tile([128, NT, D], F32)   # LN1 output, f32
            x2T_sb = [singles.tile([128, NQ], BF16, tag=f"x2T{c}", name=f"x2T{c}") for c in range(2)]
            hT_sb = singles.tile([128, 8, NQ], BF16)   # gelu(ffn1), T layout

            NT2 = NT // 2  # 256-query tiles
            def emit_ffn(T):
                # ---- FFN for this 256-query tile (overlaps next tile's attn)
                for f in range(8):
                    fcols = slice(f * 128, (f + 1) * 128)
                    ncols = slice(256 * T, 256 * T + 256)
                    ps_h = ps_big.tile([128, 256], F32, tag="psA")
                    nc.tensor.matmul(
                        ps_h, w1_sb[0][:, fcols], x2T_sb[0][:, ncols],
                        start=True, stop=False,
                    )
                    nc.tensor.matmul(
                        ps_h, w1_sb[1][:, fcols], x2T_sb[1][:, ncols],
                        start=False, stop=True,
                    )
                    nc.scalar.activation(
                        out=hT_sb[:, f, ncols], in_=ps_h, func=AF.Gelu
                    )

                for qc in range(2):
                    tt = 2 * T + qc
                    ps_y2 = ps_big.tile([128, D], F32, tag="psA")
                    for f in range(8):
                        nc.tensor.matmul(
                            ps_y2,
                            hT_sb[:, f, tt * 128 : (tt + 1) * 128],
                            w2_sb[:, f, :],
                            start=(f == 0), stop=(f == 7),
                        )
                    x3 = attn.tile([128, D], F32, tag="x3")
                    nc.vector.tensor_add(x3, ps_y2, x2_all[:, tt, :])

                    stats2 = small.tile([128, 6], F32, tag="stats2")
                    nc.vector.bn_stats(out=stats2, in_=x3)
                    mv2 = small.tile([128, 2], F32, tag="mv2")
                    nc.vector.bn_aggr(out=mv2, in_=stats2)
                    rstd2 = small.tile([128, 1], F32, tag="rstd2")
                    nc.scalar.activation(
                        out=rstd2, in_=mv2[:, 1:2], func=AF.Sqrt, bias=eps_sb
                    )
                    nc.vector.reciprocal(rstd2, rstd2)
                    o_sb = attn.tile([128, D], F32, tag="o_sb")
                    nc.vector.tensor_scalar(
                        out=o_sb, in0=x3, scalar1=mv2[:, 0:1], scalar2=rstd2,
                        op0=mybir.AluOpType.subtract, op1=mybir.AluOpType.mult,
                    )
                    nc.sync.dma_start(
                        out=out[tt * 128 : (tt + 1) * 128, :], in_=o_sb
                    )

            for T in range(NT2):
                qcols = slice(PAD + 256 * T, PAD + 256 * T + 256)
                k0 = 256 * T  # halo row of window start

                mT = attn.tile([128, 256 + 4 * PAD], BF16, tag="maskT")
                nc.scalar.dma_start(out=mT, in_=maskb[T])

                aoT = [
                    attn.tile([128, 256], BF16, tag=f"aoT{c}", name=f"aoT{c}")
                    for c in range(2)
                ]
                ps_oo = None
                ps_sum = None
                # query j's keys sit at window offsets [j+PAD-L+1, j+PAD+L-1]
                # with doc length L <= PAD+1, so a key chunk at window offsets
                # [a, b) reaches queries j in [a-2*PAD, b):
                qA = slice(0, 128)
                qB = slice(128 - 2 * PAD, 256)
                qC = slice(256 - 2 * PAD, 256)
                for h in range(H):
                    qrows = slice((h % 2) * DH, (h % 2) * DH + DH)
                    qt = qT_sb[h // 2]
                    kt = kT_sb[h // 2]
                    qgA = slice(PAD + 256 * T + qA.start, PAD + 256 * T + qA.stop)
                    qgB = slice(PAD + 256 * T + qB.start, PAD + 256 * T + qB.stop)
                    qgC = slice(PAD + 256 * T + qC.start, PAD + 256 * T + qC.stop)
                    # S^T[k, q] packed into ONE psum bank per head:
                    # A | B | C laid side by side
                    WB = 128 + 2 * PAD
                    WC = 2 * PAD
                    ps_st = ps_st_pool.tile([128, 128 + WB + WC], F32, tag="st")
                    nc.tensor.matmul(
                        ps_st[:, 0:128], kt[qrows, k0 : k0 + 128],
                        qt[qrows, qgA], start=True, stop=True,
                    )
                    nc.tensor.matmul(
                        ps_st[:, 128 : 128 + WB], kt[qrows, k0 + 128 : k0 + 256],
                        qt[qrows, qgB], start=True, stop=True,
                    )
                    nc.tensor.matmul(
                        ps_st[:WC, 128 + WB : 128 + WB + WC],
                        kt[qrows, k0 + 256 : k0 + 256 + WC],
                        qt[qrows, qgC], start=True, stop=True,
                    )
                    # single exp + single mask-mul over the packed layout;
                    # chunk-C partitions 64:128 hold garbage that is never
                    # read downstream
                    eT = attn.tile([128, 128 + WB + WC], BF16, tag="eT")
                    nc.scalar.activation(
                        out=eT, in_=ps_st, func=AF.Exp, scale=0.125
                    )
                    nc.vector.tensor_mul(eT, eT, mT)
                    # per-pair row sums broadcast over partitions via PE ones;
                    # start=True clears the whole bank's has_written bits, so
                    # later region writes overwrite-or-accumulate correctly
                    if h % 2 == 0:
                        ps_pair = ps_acc.tile([128, 512], F32, tag="acc")
                        ps_sum = ps_pair[:, 0:256]
                        ps_oo = ps_pair[:, 256:512]
                    orows = slice((h % 2) * DH, (h % 2) * DH + DH)
                    nc.tensor.matmul(
                        ps_sum[orows, qA], ones_sb[:, :DH], eT[:, 0:128],
                        start=True, stop=False,
                    )
                    nc.tensor.matmul(
                        ps_sum[orows, qB], ones_sb[:, :DH], eT[:, 128 : 128 + WB],
                        start=False, stop=False,
                    )
                    nc.tensor.matmul(
                        ps_sum[orows, qC], ones_sb[:WC, :DH],
                        eT[:WC, 128 + WB : 128 + WB + WC],
                        start=False, stop=True,
                    )
                    # attn_out^T[dh, q] = sum_k v[k, dh] * E^T[k, q] (unnorm.)
                    hc = slice(h * DH, (h + 1) * DH)
                    nc.tensor.matmul(
                        ps_oo[orows, qA], v_sb[:, 2 * T, hc], eT[:, 0:128],
                        start=True, stop=False,
                    )
                    nc.tensor.matmul(
                        ps_oo[orows, qB], v_sb[:, 2 * T + 1, hc],
                        eT[:, 128 : 128 + WB],
                        start=False, stop=False,
                    )
                    nc.tensor.matmul(
                        ps_oo[orows, qC], v_sb[:WC, 2 * T + 2, hc],
                        eT[:WC, 128 + WB : 128 + WB + WC],
                        start=False, stop=True,
                    )
                    if h % 2 == 1:
                        rbpair = attn.tile([128, 256], F32, tag="rbpair")
                        nc.vector.reciprocal(rbpair, ps_sum)
                        # normalize + downcast to SBUF in one DVE op
                        nc.vector.tensor_mul(aoT[h // 2], ps_oo, rbpair)

                for qc in range(2):
                    tt = 2 * T + qc
                    qsl = slice(qc * 128, qc * 128 + 128)
                    # out-projection: y[q, d] = sum_din aoT[din, q]*outw[din, d]
                    ps_y = ps_big.tile([128, D], F32, tag="psA")
                    nc.tensor.matmul(
                        ps_y, aoT[0][:, qsl], outw_sb[0], start=True, stop=False
                    )
                    nc.tensor.matmul(
                        ps_y, aoT[1][:, qsl], outw_sb[1], start=False, stop=True
                    )

                    # residual + LN1 (f32)
                    x_sb = attn.tile([128, D], F32, tag="x_sb")
                    nc.sync.dma_start(
                        out=x_sb, in_=x_own[tt * 128 : (tt + 1) * 128, :]
                    )
                    x1 = attn.tile([128, D], F32, tag="x1")
                    nc.vector.tensor_add(x1, ps_y, x_sb)

                    stats = small.tile([128, 6], F32, tag="stats")
                    nc.vector.bn_stats(out=stats, in_=x1)
                    mv = small.tile([128, 2], F32, tag="mv")
                    nc.vector.bn_aggr(out=mv, in_=stats)
                    rstd = small.tile([128, 1], F32, tag="rstd")
                    nc.scalar.activation(
                        out=rstd, in_=mv[:, 1:2], func=AF.Sqrt, bias=eps_sb
                    )
                    nc.vector.reciprocal(rstd, rstd)
                    x2 = x2_all[:, tt, :]
                    nc.vector.tensor_scalar(
                        out=x2, in0=x1, scalar1=mv[:, 0:1], scalar2=rstd,
                        op0=mybir.AluOpType.subtract, op1=mybir.AluOpType.mult,
                    )
                    # transpose x2 (f32) directly into x2T chunks
                    ps_t2 = ps_big.tile([128, 2, 128], F32, tag="psA")
                    for c in range(2):
                        nc.tensor.transpose(
                            ps_t2[:, c, :], x2[:, c * 128 : (c + 1) * 128], identf
                        )
                    nc.vector.tensor_copy(
                        out=x2T_sb[0][:, tt * 128 : (tt + 1) * 128],
                        in_=ps_t2[:, 0, :],
                    )
                    nc.scalar.copy(
                        out=x2T_sb[1][:, tt * 128 : (tt + 1) * 128],
                        in_=ps_t2[:, 1, :],
                    )

            for T in range(NT2):
                emit_ffn(T)

    _split_excess_waits(nc)
    return nc


def _host_prep(x, pulse_to_dom_idx, qkv_w, out_w, ff_w1, ff_w2):
    bf = ml_dtypes.bfloat16
    dom = np.asarray(pulse_to_dom_idx)
    # document segments must fit the 64-row halo
    _, counts = np.unique(dom, return_counts=True)
    assert counts.max() <= PAD + 1, f"doc segment too long: {counts.max()}"

    xpad = np.zeros((N + 2 * PAD, D), np.float32)
    xpad[PAD : PAD + N] = x
    dompad = np.full(N + 2 * PAD, -1, np.int64)
    dompad[PAD : PAD + N] = dom

    in_maps = []
    for c in range(NCORES):
        h0 = c * NQ  # padded-row index of halo start
        xT_c = np.ascontiguousarray(xpad[h0 : h0 + HALO].T.astype(bf))
        x_own_c = np.ascontiguousarray(xpad[h0 + PAD : h0 + PAD + NQ])
        WB = 128 + 2 * PAD
        WC = 2 * PAD
        mb = np.zeros((NT // 2, 128, 256 + 4 * PAD), bf)
        for T in range(NT // 2):
            qs = h0 + PAD + 256 * T          # padded idx of first query row
            ks = h0 + 256 * T                # padded idx of window start
            # transposed mask [key, query], packed to match eT layout:
            # A: keys 0:128 x q 0:128 | B: keys 128:256 x q 128-2PAD:256 |
            # C: keys 256:256+WC x q 256-WC:256
            same = (dompad[ks : ks + 256 + WC, None]
                    == dompad[None, qs : qs + 256])
            mb[T, :, 0:128] = same[0:128, 0:128].astype(bf)
            mb[T, :, 128 : 128 + WB] = same[128:256, 256 - WB : 256].astype(bf)
            mb[T, :WC, 128 + WB :] = same[256 : 256 + WC, 256 - WC :].astype(bf)
        in_maps.append(
            {
                "xT": xT_c,
                "x_own": x_own_c,
                "qkvw": np.ascontiguousarray(qkv_w.astype(bf)),
                "outw": np.ascontiguousarray(out_w.astype(bf)),
                "w1": np.ascontiguousarray(ff_w1.astype(bf)),
                "w2": np.ascontiguousarray(ff_w2.astype(bf)),
                "maskb": mb,
            }
        )
    return in_maps


def _make_runner(nc, n_cores=NCORES):
    """Compile the bass module via the bass2jax/PJRT path once and return
    (prep, execute, collect): prep(in_maps) -> flat input list,
    execute(flat) -> jax out arrays, collect(outs) -> full output."""
    import jax
    from jax.sharding import Mesh, PartitionSpec
    from jax.experimental.shard_map import shard_map
    from concourse import bass2jax as b2j

    b2j.install_neuronx_cc_hook()

    partition_name = nc.partition_id_tensor.name if nc.partition_id_tensor else None
    in_names, out_names, out_avals, zero_outs = [], [], [], []
    for alloc in nc.m.functions[0].allocations:
        if not isinstance(alloc, mybir.MemoryLocationSet):
            continue
        name = alloc.memorylocations[0].name
        if alloc.kind == "ExternalInput":
            if name != partition_name:
                in_names.append(name)
        elif alloc.kind == "ExternalOutput":
            out_names.append(name)
            shape = tuple(alloc.tensor_shape)
            dtype = mybir.dt.np(alloc.dtype)
            out_avals.append(jax.core.ShapedArray(shape, dtype))
            zero_outs.append(np.zeros(shape, dtype))
    n_params = len(in_names)
    n_outs = len(out_avals)
    all_in_names = list(in_names) + list(out_names)
    if partition_name is not None:
        all_in_names.append(partition_name)
    donate = tuple(range(n_params, n_params + n_outs))

    def _body(*args):
        operands = list(args)
        if partition_name is not None:
            operands.append(b2j.partition_id_tensor())
        outs = b2j._bass_exec_p.bind(
            *operands,
            out_avals=tuple(out_avals),
            in_names=tuple(all_in_names),
            out_names=tuple(out_names),
            lowering_input_output_aliases=(),
            sim_require_finite=True,
            sim_require_nnan=True,
            nc=nc,
        )
        return tuple(outs)

    devices = jax.devices()[:n_cores]
    mesh = Mesh(np.asarray(devices), ("core",))
    in_specs = (PartitionSpec("core"),) * (n_params + n_outs)
    out_specs = (PartitionSpec("core"),) * n_outs
    sharded = jax.jit(
        shard_map(
            _body, mesh=mesh, in_specs=in_specs, out_specs=out_specs,
            check_rep=False,
        ),
        donate_argnums=donate, keep_unused=True,
    )

    def prep(in_maps):
        per_core = [[np.asarray(m[name]) for name in in_names] for m in in_maps]
        flat = [
            np.concatenate([per_core[c][i] for c in range(n_cores)], axis=0)
            for i in range(n_params)
        ]
        return flat

    def execute(flat):
        concat_zeros = [
            np.zeros((n_cores * z.shape[0], *z.shape[1:]), z.dtype)
            for z in zero_outs
        ]
        outs = sharded(*flat, *concat_zeros)
        jax.block_until_ready(outs)
        return outs

    def collect(outs):
        # single output "out": [n_cores*NQ, D] -> full
        return np.asarray(outs[0])

    return prep, execute, collect


def _get_runner():
    if "runner" not in _CACHE:
        _CACHE["nc"] = _build_bass()
        _CACHE["runner"] = _make_runner(_CACHE["nc"])
    return _CACHE["runner"]


def kernel(
    x, pulse_to_dom_idx, qkv_w, qkv_b, out_w, out_b,
    ff_w1, ff_b1, ff_w2, ff_b2, ln1_g, ln1_b, ln2_g, ln2_b,
):
    x = np.asarray(x, np.float32)
    for b in (qkv_b, out_b, ff_b1, ff_b2, ln1_b, ln2_b):
        assert np.abs(np.asarray(b)).max() == 0.0, "nonzero bias unsupported"
    for g in (ln1_g, ln2_g):
        assert np.abs(np.asarray(g) - 1.0).max() == 0.0, "ln gain unsupported"

    prep, execute, collect = _get_runner()
    in_maps = _host_prep(
        x, pulse_to_dom_idx,
        np.asarray(qkv_w, np.float32), np.asarray(out_w, np.float32),
        np.asarray(ff_w1, np.float32), np.asarray(ff_w2, np.float32),
    )
    flat = prep(in_maps)
    try:
        outs = execute(flat)
    except Exception:
        # transient NRT_EXEC_UNIT_UNRECOVERABLE has been observed on the
        # first execution after a device teardown; one retry clears it
        time.sleep(2.0)
        outs = execute(flat)
    return collect(outs)

